# revision 40
# baseline (speedup 1.0000x reference)
# DGCNN graph-feature module on 8 Trainium2 NeuronCores.
#
# Data-parallel over batch B=8 (one batch element per core). Per core:
#   - score matrix nd[i,j] = p_i.p_j - |p_j|^2/2 - |p_i|^2/2 via one bf16
#     PE matmul over a fused [66, N] operand (rows: points, ones, -|p|^2/2);
#     the extra per-row constant does not change row-wise top-k
#   - exact top-16 per row via DVE max8 / max_index / match_replace (f32)
#   - gather of G = W1a p^T in a channel-packed layout: Gt4[16k+j, m, c4]
#     = G[4j+c4, m] replicated across the 8 Q7 cores, so one ap_gather
#     instruction (d=4) gathers for all 8 point-blocks at once with its
#     per-core index lists -- 8x fewer SBUF read commands than d=1
#   - gather output converted to fold-2 channel-major h1 via block-diag
#     one-hot PE matmuls; DVE tensor_tensor_reduce fuses the PSUM drain
#     with the +Cc add and the BN1 sum; ACT Square passes accumulate sumsq
#   - exact BatchNorm batch stats across all 8 cores via tiny AllReduces
#   - z = relu(a1 h1 + b1) on ACT, h2 = W2 z via a block-diagonal [128,128]
#     matmul, max-pool over K direct from PSUM, BN2 folded into the final
#     affine (max-pool commutes with BN2+ReLU since the scale is positive)
import numpy as np
from contextlib import ExitStack

import concourse.bass as bass
from concourse import bacc, library_config
import concourse.tile as tile
from concourse import mybir
from concourse.bass_utils import run_bass_kernel_spmd

B, N, C, K = 8, 4096, 64, 16
NB = N // 128                      # 32 row-blocks of 128 points
M_TOTAL = float(B * N * K)         # BN sample count over the whole batch
EPS = 1e-5
NEG_BIG = -1e30
F32 = mybir.dt.float32
BF16 = mybir.dt.bfloat16
I16 = mybir.dt.int16
U32 = mybir.dt.uint32
AF = mybir.ActivationFunctionType
ALU = mybir.AluOpType
AX = mybir.AxisListType

_NC_CACHE = {}


def build_nc(n_cores=8, use_collectives=True, use_gather=True,
             use_ttr=False, inplace_mr=True, stages='ABDE'):
    nc = bacc.Bacc("TRN2", target_bir_lowering=False, debug=False, num_devices=n_cores)
    pts = nc.declare_dram_parameter("pts", [N, C], F32, isOutput=False)
    w1a4 = nc.declare_dram_parameter("w1a4", [C, 512], F32, isOutput=False)
    w1cT2 = nc.declare_dram_parameter("w1cT2", [C, 256], F32, isOutput=False)
    w2blk = nc.declare_dram_parameter("w2blk", [128, 128], BF16, isOutput=False)
    w2T32 = nc.declare_dram_parameter("w2T32", [C, C], F32, isOutput=False)
    e4up0 = nc.declare_dram_parameter("e4up0", [128, 512], BF16, isOutput=False)
    e4up1 = nc.declare_dram_parameter("e4up1", [128, 512], BF16, isOutput=False)
    self64 = nc.declare_dram_parameter("self64", [128, C], F32, isOutput=False)
    repl64 = nc.declare_dram_parameter("repl64", [C, 128], F32, isOutput=False)
    gb = nc.declare_dram_parameter("gb", [C, 4], F32, isOutput=False)
    out_t = nc.declare_dram_parameter("out_t", [128, 2048], F32, isOutput=True)

    group = [list(range(n_cores))]

    with tile.TileContext(nc) as tc:
      with ExitStack() as ctx:
        per = ctx.enter_context(tc.tile_pool(name="per", bufs=1))
        small = ctx.enter_context(tc.tile_pool(name="small", bufs=3))
        dram = ctx.enter_context(tc.tile_pool(name="dram", bufs=1, space="DRAM"))

        # ---- persistent SBUF tensors
        pT_lhs_t = [per.tile([C + 1, 512], mybir.dt.float32r,
                             name=f"pTl{j}", tag=f"pTl{j}")
                    for j in range(8)]
        pT_rhs_t = [per.tile([C + 1, 512], mybir.dt.float32r,
                             name=f"pTr{j}", tag=f"pTr{j}")
                    for j in range(8)]
        Gt4 = per.tile([128, N * 4], BF16)   # [128, m, c4]: part 16k+j = ch 4j+c4
        CcT2 = per.tile([128, N // 2], BF16)  # part c+64s = Cc[c, 512*(2kp+s)+m]
        h1_2 = per.tile([128, N * K // 2], BF16)  # fold-2 channel-major h1
        pooled = per.tile([128, N // 2], BF16)
        s1a = per.tile([128, 48], F32)
        s1b = per.tile([128, 16], F32)
        q1a = per.tile([128, 48], F32)
        q1b = per.tile([128, 16], F32)
        s2a = per.tile([128, 28], F32)
        s2b = per.tile([128, 4], F32)
        q2a = per.tile([128, 56], F32)
        q2b = per.tile([128, 8], F32)

        # ---- DRAM collective bounce buffers
        cc1_in = dram.tile([C, 2], F32)
        cc1_out = dram.tile([C, 2], F32)
        cc1b_in = dram.tile([C, 2], F32)
        cc1b_out = dram.tile([C, 2], F32)
        cc2_in = dram.tile([C, 2], F32)
        cc2_out = dram.tile([C, 2], F32)
        cc2b_in = dram.tile([C, 2], F32)
        cc2b_out = dram.tile([C, 2], F32)

        # ---- constants / weights
        identity = per.tile([128, 128], F32)
        ones128 = per.tile([128, 128], F32)
        nc.vector.memset(ones128, 1.0)
        nc.gpsimd.affine_select(
            identity, ones128, pattern=[[-1, 128]], compare_op=ALU.is_equal,
            fill=0.0, base=0, channel_multiplier=1,
        )
        ones_col = per.tile([C, 1], F32)
        nc.vector.memset(ones_col, 1.0)
        eps_col = per.tile([C, 1], F32)
        nc.vector.memset(eps_col, EPS)

        w1a4_s = per.tile([C, 512], F32)
        nc.sync.dma_start(out=w1a4_s, in_=w1a4[:, :])
        w1cT2_s = per.tile([C, 256], F32)
        nc.sync.dma_start(out=w1cT2_s, in_=w1cT2[:, :])
        w2blk_s = per.tile([128, 128], BF16)
        nc.sync.dma_start(out=w2blk_s, in_=w2blk[:, :])
        w2T32_s = per.tile([C, C], F32)
        nc.sync.dma_start(out=w2T32_s, in_=w2T32[:, :])
        e4up0_s = per.tile([128, 512], BF16)
        nc.sync.dma_start(out=e4up0_s, in_=e4up0[:, :])
        e4up1_s = per.tile([128, 512], BF16)
        nc.sync.dma_start(out=e4up1_s, in_=e4up1[:, :])
        self64_s = per.tile([128, C], F32)
        nc.sync.dma_start(out=self64_s, in_=self64[:, :])
        repl64_s = per.tile([C, 128], F32)
        nc.sync.dma_start(out=repl64_s, in_=repl64[:, :])
        gb_s = per.tile([C, 4], F32)
        nc.sync.dma_start(out=gb_s, in_=gb[:, :])

        # ================= PHASE A: transpose points, norms, Gt4, CcT2 ====
        with tc.tile_pool(name="psA", bufs=2, space="PSUM") as psA, \
             tc.tile_pool(name="ldA", bufs=3) as ldA, \
             tc.tile_pool(name="sqA", bufs=2) as sqA:
            for t in range(NB):
                sl = slice(t * 128, (t + 1) * 128)
                pt_tile = ldA.tile([128, C], F32)
                nc.sync.dma_start(out=pt_tile, in_=pts[sl, :])
                ps_tr = psA.tile([C, 128], F32)
                nc.tensor.transpose(ps_tr, pt_tile, identity)
                cj, co = t // 4, (t % 4) * 128
                nc.scalar.activation(
                    pT_lhs_t[cj][0:C, co:co + 128], ps_tr, AF.Copy)
                nc.vector.tensor_copy(
                    pT_rhs_t[cj][0:C, co:co + 128], ps_tr)
            # pT_rhs row 64: -|p|^2/2 via elementwise square + ones-matmul
            # pT_lhs row 64: ones (ACT writes f32r; memset can't)
            for j in range(N // 512):
                sqt = sqA.tile([C, 512], F32, tag="sqt")
                nc.vector.tensor_mul(sqt, pT_rhs_t[j][0:C, :].bitcast(F32),
                                     pT_rhs_t[j][0:C, :].bitcast(F32))
                ps_row = psA.tile([1, 512], F32, tag="psrow")
                nc.tensor.matmul(ps_row, lhsT=ones_col, rhs=sqt,
                                 start=True, stop=True)
                nc.scalar.activation(pT_rhs_t[j][C:C + 1, :], ps_row, AF.Copy,
                                     scale=-0.5)
                nc.scalar.activation(pT_lhs_t[j][C:C + 1, :], ps_row, AF.Copy,
                                     scale=0.0, bias=1.0)

        nc.gpsimd.load_library(library_config.ap_gather)

        def fold_stats(scols, qcols, ps_pool, cc_corr=False,
                       ps_tag="ps_st"):
            s = small.tile([128, 1], F32, tag="s_red")
            nc.vector.reduce_sum(out=s, in_=scols, axis=AX.X)
            if cc_corr:
                ccs = small.tile([128, 1], F32, tag="ccs")
                nc.vector.reduce_sum(out=ccs, in_=CcT2, axis=AX.X)
                s2t = small.tile([128, 1], F32, tag="s_corr")
                nc.vector.tensor_scalar_mul(s2t, ccs, float(K))
                sc = small.tile([128, 1], F32, tag="s_corrd")
                nc.vector.tensor_add(sc, s, s2t)
                s = sc
            qq = small.tile([128, 1], F32, tag="q_red")
            nc.vector.reduce_sum(out=qq, in_=qcols, axis=AX.X)
            sq = small.tile([128, 2], F32, tag="sq_pack")
            nc.vector.tensor_copy(sq[:, 0:1], s)
            nc.vector.tensor_copy(sq[:, 1:2], qq)
            ps_st = ps_pool.tile([C, 2], F32, tag=ps_tag)
            nc.tensor.matmul(ps_st, lhsT=self64_s, rhs=sq,
                             start=True, stop=True)
            st = small.tile([C, 2], F32, tag="st_sb")
            nc.scalar.activation(st, ps_st, AF.Copy)
            return st

        # ================= PHASE B: scores, top-16, gather, h1, stats1 ====
        # Row-blocks processed set-major: set q covers blocks {4k+q}.
        gview = Gt4.rearrange("p (m c) -> p m c", c=4)

        def emit_set_blocks(q, psB, psIdx, ndb, idxb, topb):
            idxf4 = idxb.tile([128, 128], F32, tag="idxf4")
            for k in range(8):
                t = 4 * k + q
                sl = slice(t * 128, (t + 1) * 128)
                ndf = ndb.tile([128, N], F32, tag="ndf")
                lcj, lco = t // 4, (t % 4) * 128
                lhsT_blk = pT_lhs_t[lcj][:, lco:lco + 128]
                for j in range(4):
                    ps_nd = psB.tile([128, 1024], F32)
                    for jj in range(2):
                        nc.tensor.matmul(
                            ps_nd[:, jj * 512:(jj + 1) * 512],
                            lhsT=lhsT_blk, rhs=pT_rhs_t[2 * j + jj][:, :],
                            start=True, stop=True)
                    nc.scalar.activation(
                        ndf[:, j * 1024:(j + 1) * 1024], ps_nd, AF.Copy)
                m8a = topb.tile([128, 8], F32, tag="m8a")
                nc.vector.max(out=m8a, in_=ndf)
                i8a = topb.tile([128, 8], U32, tag="i8a")
                nc.vector.max_index(out=i8a, in_max=m8a, in_values=ndf)
                if inplace_mr:
                    ndp = ndf
                else:
                    ndp = ndb.tile([128, N], F32, tag="ndp")
                nc.vector.match_replace(out=ndp, in_to_replace=m8a,
                                        in_values=ndf, imm_value=NEG_BIG)
                m8b = topb.tile([128, 8], F32, tag="m8b")
                nc.vector.max(out=m8b, in_=ndp)
                i8b = topb.tile([128, 8], U32, tag="i8b")
                nc.vector.max_index(out=i8b, in_max=m8b, in_values=ndp)
                co = 16 * k
                nc.vector.tensor_copy(idxf4[:, co:co + 8], i8a)
                nc.vector.tensor_copy(idxf4[:, co + 8:co + 16], i8b)
            nc.tensor.transpose(psIdx, idxf4, identity)

        with tc.tile_pool(name="psB", bufs=2, space="PSUM") as psB, \
             tc.tile_pool(name="psI", bufs=2, space="PSUM") as psI, \
             tc.tile_pool(name="psC2", bufs=2, space="PSUM") as psC2, \
             tc.tile_pool(name="ndb", bufs=2 if inplace_mr else 1) as ndb, \
             tc.tile_pool(name="idxb", bufs=2) as idxb, \
             tc.tile_pool(name="topb", bufs=2) as topb, \
             tc.tile_pool(name="i16b", bufs=2) as i16b, \
             tc.tile_pool(name="ghb", bufs=3) as ghb, \
             tc.tile_pool(name="sqjb", bufs=2) as sqjb:

            def emit_set_consume(q, gh_tiles):
                # conversion matmuls + fused drain/add/sum + sumsq, set q
                for h in range(4):
                    ght = gh_tiles[h]
                    ghv = ght.rearrange("p (m c) -> p m c", c=4)
                    pbase = 128 * q + 32 * h
                    for kp in range(4):
                        a, up = kp // 2, kp % 2
                        e4_s = e4up0_s if up == 0 else e4up1_s
                        ps_h1 = psC2.tile([128, 512], F32)
                        for c4 in range(4):
                            nc.tensor.matmul(
                                ps_h1,
                                lhsT=e4_s[64 * a:64 * a + 64,
                                          c4 * 128:(c4 + 1) * 128],
                                rhs=ghv[64 * a:64 * a + 64, :, c4],
                                start=(c4 == 0), stop=(c4 == 3))
                        fo = kp * 8192 + pbase * 16
                        col = q * 16 + h * 4 + kp
                        if col < 48:
                            s1t, s1c = s1a, col
                            q1t, q1c = q1a, col
                        else:
                            s1t, s1c = s1b, col - 48
                            q1t, q1c = q1b, col - 48
                        h1v = h1_2[:, fo:fo + 512].rearrange(
                            "p (m kk) -> p m kk", kk=16)
                        ccv = CcT2[:, kp * 512 + pbase:kp * 512 + pbase + 32]
                        ccb = ccv.rearrange("p (m o) -> p m o", o=1)\
                            .to_broadcast([128, 32, 16])
                        if use_ttr:
                            nc.vector.tensor_tensor_reduce(
                                out=h1v,
                                in0=ps_h1.rearrange("p (m kk) -> p m kk", kk=16),
                                in1=ccb, scale=1.0, scalar=0.0,
                                op0=ALU.add, op1=ALU.add,
                                accum_out=s1t[:, s1c:s1c + 1],
                                opt_aps=False)
                        else:
                            # drain w/ sum of pre-add h1'; Cc-part of the
                            # sum is added analytically in fold_stats
                            slab = sqjb.tile([128, 512], BF16, tag="slab")
                            nc.scalar.activation(
                                slab, ps_h1, AF.Copy,
                                accum_out=s1t[:, s1c:s1c + 1])
                            nc.vector.tensor_add(
                                h1v,
                                slab.rearrange("p (m kk) -> p m kk", kk=16),
                                ccb)
                        sqj = sqjb.tile([128, 512], BF16, tag="sqj")
                        nc.scalar.activation(
                            sqj, h1_2[:, fo:fo + 512], AF.Square,
                            accum_out=q1t[:, q1c:q1c + 1])

            if 'B' not in stages:
                nc.vector.memset(s1a, 1.0)
                nc.vector.memset(s1b, 1.0)
                nc.vector.memset(q1a, 2.0)
                nc.vector.memset(q1b, 2.0)
                nc.vector.memset(h1_2, 0.125)
            def emit_gcc_prep(ps_pool):
                # Gt4[16k+j, m, c4] = G[4j+c4, m], replicated over k
                gv = Gt4.rearrange("p (m c) -> p m c", c=4)
                for c4 in range(4):
                    for j in range(N // 512):
                        ps_g = ps_pool.tile([128, 512], F32, tag="ps_h1")
                        nc.tensor.matmul(
                            ps_g, lhsT=w1a4_s[:, c4 * 128:(c4 + 1) * 128],
                            rhs=pT_lhs_t[j][0:C, :].bitcast(F32),
                            start=True, stop=True)
                        nc.scalar.activation(
                            gv[:, j * 512:(j + 1) * 512, c4], ps_g, AF.Copy)
                # CcT2[c+64s, kp*512+m] = Cc[c, 512*(2kp+s)+m]
                for kp in range(4):
                    ps_c = ps_pool.tile([128, 512], F32, tag="ps_h1")
                    for s in range(2):
                        blk = 2 * kp + s
                        nc.tensor.matmul(
                            ps_c, lhsT=w1cT2_s[:, s * 128:(s + 1) * 128],
                            rhs=pT_lhs_t[blk][0:C, :].bitcast(F32),
                            start=(s == 0), stop=(s == 1))
                    nc.vector.tensor_copy(
                        CcT2[:, kp * 512:(kp + 1) * 512], ps_c)

            if 'B' not in stages:
                emit_gcc_prep(psC2)
            prev = None
            for q in range(4 if 'B' in stages else 0):
                psIdx = psI.tile([128, 128], F32)
                emit_set_blocks(q, psB, psIdx, ndb, idxb, topb)
                idx16 = i16b.tile([128, 128], I16)
                nc.vector.tensor_copy(idx16, psIdx)
                if q == 0:
                    emit_gcc_prep(psC2)
                gh_tiles = []
                for h in range(4):
                    ght = ghb.tile([128, 512 * 4], BF16, tag="gh")
                    if use_gather:
                        nc.gpsimd.ap_gather(
                            out_ap=ght.rearrange("p (m c) -> p m c", c=4),
                            in_ap=gview, idxs_ap=idx16[:, 32 * h:32 * h + 32],
                            channels=128, num_elems=N, d=4, num_idxs=512,
                        )
                    else:
                        nc.vector.memset(ght, 0.25)
                    gh_tiles.append(ght)
                if prev is not None:
                    emit_set_consume(prev[0], prev[1])
                    if prev[0] == 2 and use_collectives and 'B' in stages:
                        # partial stats over sets 0-2 allreduced while
                        # set 3 computes (sums split additively)
                        st_a = fold_stats(s1a, q1a, psI,
                                          cc_corr=not use_ttr,
                                          ps_tag="psIdx")
                        nc.sync.dma_start(out=cc1_in[:], in_=st_a)
                        nc.gpsimd.collective_compute(
                            "AllReduce", ALU.add, replica_groups=group,
                            ins=[cc1_in[:].opt()], outs=[cc1_out[:].opt()],
                        )
                prev = (q, gh_tiles)
            if prev is not None:
                emit_set_consume(prev[0], prev[1])

        # ================= PHASE C: stats1 allreduce -> a1, b1 ============

        def stats_to_affine(st, g_col, b_col):
            mean = small.tile([C, 1], F32, tag="mean")
            nc.vector.tensor_scalar_mul(mean, st[:, 0:1], 1.0 / M_TOTAL)
            ex2 = small.tile([C, 1], F32, tag="ex2")
            nc.vector.tensor_scalar_mul(ex2, st[:, 1:2], 1.0 / M_TOTAL)
            m2 = small.tile([C, 1], F32, tag="m2")
            nc.vector.tensor_mul(m2, mean, mean)
            var = small.tile([C, 1], F32, tag="var")
            nc.vector.tensor_sub(var, ex2, m2)
            sd = small.tile([C, 1], F32, tag="sd")
            nc.scalar.activation(sd, var, AF.Sqrt, bias=eps_col)
            rs = small.tile([C, 1], F32, tag="rs")
            nc.vector.reciprocal(rs, sd)
            a = small.tile([C, 1], F32, tag="a_aff")
            nc.vector.tensor_mul(a, g_col, rs)
            tmp = small.tile([C, 1], F32, tag="tmp_aff")
            nc.vector.tensor_mul(tmp, mean, a)
            b = small.tile([C, 1], F32, tag="b_aff")
            nc.vector.tensor_sub(b, b_col, tmp)
            ab = small.tile([C, 2], F32, tag="ab_pack")
            nc.vector.tensor_copy(ab[:, 0:1], a)
            nc.vector.tensor_copy(ab[:, 1:2], b)
            return ab

        def replicate_ab(ab, ps_pool):
            ps_ab = ps_pool.tile([128, 2], F32, tag="ps_ab")
            nc.tensor.matmul(ps_ab, lhsT=repl64_s, rhs=ab,
                             start=True, stop=True)
            ab2 = small.tile([128, 2], F32, tag="ab2")
            nc.scalar.activation(ab2, ps_ab, AF.Copy)
            return ab2

        with tc.tile_pool(name="psCD", bufs=1, space="PSUM") as psCD, \
             tc.tile_pool(name="psD2", bufs=3, space="PSUM") as psD2, \
             tc.tile_pool(name="zb", bufs=3) as zb, \
             tc.tile_pool(name="sqj2", bufs=2) as sqj2, \
             tc.tile_pool(name="outb", bufs=2) as outb:
            if use_collectives and 'B' in stages:
                st_b = fold_stats(s1b, q1b, psCD, cc_corr=False)
                nc.sync.dma_start(out=cc1b_in[:], in_=st_b)
                nc.gpsimd.collective_compute(
                    "AllReduce", ALU.add, replica_groups=group,
                    ins=[cc1b_in[:].opt()], outs=[cc1b_out[:].opt()],
                )
                st1a = small.tile([C, 2], F32, tag="st1a")
                nc.sync.dma_start(out=st1a, in_=cc1_out[:])
                st1b = small.tile([C, 2], F32, tag="st1b")
                nc.sync.dma_start(out=st1b, in_=cc1b_out[:])
                st1r = small.tile([C, 2], F32, tag="st1r")
                nc.vector.tensor_add(st1r, st1a, st1b)
            else:
                stx = fold_stats(s1a, q1a, psCD, cc_corr=not use_ttr)
                sty = fold_stats(s1b, q1b, psCD, cc_corr=False)
                st1 = small.tile([C, 2], F32, tag="st1f")
                nc.vector.tensor_add(st1, stx, sty)
                nc.sync.dma_start(out=cc1_in[:], in_=st1)
                nc.sync.dma_start(out=cc1_out[:], in_=cc1_in[:])
                st1r = small.tile([C, 2], F32, tag="st1r")
                nc.sync.dma_start(out=st1r, in_=cc1_out[:])
            ab1 = stats_to_affine(st1r, gb_s[:, 0:1], gb_s[:, 1:2])
            ab1_2 = replicate_ab(ab1, psCD)

            def emit_stats2(s_t, q_t, cc_in_t, cc_out_t):
                zsum = small.tile([128, 1], F32, tag="zsum")
                nc.vector.reduce_sum(out=zsum, in_=s_t, axis=AX.X)
                q2sum = small.tile([128, 1], F32, tag="q2sum")
                nc.vector.reduce_sum(out=q2sum, in_=q_t, axis=AX.X)
                zq = small.tile([128, 2], F32, tag="zq_pack")
                nc.vector.tensor_copy(zq[:, 0:1], zsum)
                nc.vector.tensor_copy(zq[:, 1:2], q2sum)
                ps_zq = psCD.tile([C, 2], F32, tag="ps_zq")
                nc.tensor.matmul(ps_zq, lhsT=self64_s, rhs=zq,
                                 start=True, stop=True)
                zqc = small.tile([C, 2], F32, tag="zqc")
                nc.scalar.activation(zqc, ps_zq, AF.Copy)
                ps_s2 = psCD.tile([C, 1], F32, tag="ps_s2")
                nc.tensor.matmul(ps_s2, lhsT=w2T32_s, rhs=zqc[:, 0:1],
                                 start=True, stop=True)
                st2 = small.tile([C, 2], F32, tag="st2")
                nc.scalar.activation(st2[:, 0:1], ps_s2, AF.Copy)
                nc.vector.tensor_copy(st2[:, 1:2], zqc[:, 1:2])
                nc.sync.dma_start(out=cc_in_t[:], in_=st2)
                if use_collectives:
                    nc.gpsimd.collective_compute(
                        "AllReduce", ALU.add, replica_groups=group,
                        ins=[cc_in_t[:].opt()], outs=[cc_out_t[:].opt()],
                    )
                else:
                    nc.sync.dma_start(out=cc_out_t[:], in_=cc_in_t[:])


            # ============= PHASE D: z=relu(a1 h1+b1), h2=W2blk z, pool ====
            if 'D' not in stages:
                nc.vector.memset(pooled, 0.5)
                nc.vector.memset(s2a, 1.0)
                nc.vector.memset(s2b, 1.0)
                nc.vector.memset(q2a, 2.0)
                nc.vector.memset(q2b, 2.0)
            for i in range(32 if 'D' in stages else 0):
                zt = zb.tile([128, 1024], BF16, tag="zt")
                nc.scalar.activation(
                    zt, h1_2[:, i * 1024:(i + 1) * 1024], AF.Relu,
                    scale=ab1_2[:, 0:1], bias=ab1_2[:, 1:2],
                    accum_out=(s2a[:, i:i + 1] if i < 28
                               else s2b[:, i - 28:i - 27]))
                for jj in range(2):
                    ti = 2 * i + jj
                    ps_h2 = psD2.tile([128, 512], F32, tag="ps_h2")
                    nc.tensor.matmul(ps_h2, lhsT=w2blk_s,
                                     rhs=zt[:, jj * 512:(jj + 1) * 512],
                                     start=True, stop=True)
                    nc.vector.reduce_max(
                        out=pooled[:, ti * 32:(ti + 1) * 32],
                        in_=ps_h2.rearrange("p (m kk) -> p m kk", kk=16),
                        axis=AX.X)
                    sqj = sqj2.tile([128, 512], BF16, tag="sqj2")
                    nc.scalar.activation(
                        sqj, ps_h2, AF.Square,
                        accum_out=(q2a[:, ti:ti + 1] if ti < 56
                                   else q2b[:, ti - 56:ti - 55]))
                if i == 27:
                    emit_stats2(s2a, q2a, cc2_in, cc2_out)

            # ============= PHASE E: stats2 allreduce -> final =============
            emit_stats2(s2b, q2b, cc2b_in, cc2b_out)
            st2a = small.tile([C, 2], F32, tag="st2a")
            nc.sync.dma_start(out=st2a, in_=cc2_out[:])
            st2b = small.tile([C, 2], F32, tag="st2b")
            nc.sync.dma_start(out=st2b, in_=cc2b_out[:])
            st2r = small.tile([C, 2], F32, tag="st2r")
            nc.vector.tensor_add(st2r, st2a, st2b)
            ab2 = stats_to_affine(st2r, gb_s[:, 2:3], gb_s[:, 3:4])
            ab2_2 = replicate_ab(ab2, psCD)

            for j in range(4):
                js = slice(j * 512, (j + 1) * 512)
                ob = outb.tile([128, 512], F32, tag="ob")
                nc.scalar.activation(ob, pooled[:, js], AF.Relu,
                                     scale=ab2_2[:, 0:1], bias=ab2_2[:, 1:2])
                nc.sync.dma_start(out=out_t[:, js], in_=ob)

    nc.finalize()
    return nc


def _get_nc(n_cores=8):
    if n_cores not in _NC_CACHE:
        _NC_CACHE[n_cores] = build_nc(n_cores)
    return _NC_CACHE[n_cores]


def make_in_maps(points, W1, gamma1, beta1, W2, gamma2, beta2, n_cores=8):
    import ml_dtypes
    bf = ml_dtypes.bfloat16
    pts = np.ascontiguousarray(np.asarray(points, np.float32))
    W1 = np.asarray(W1, np.float32)
    W2 = np.asarray(W2, np.float32)
    W1a = W1[:, :C]
    W1c = W1[:, C:] - W1a

    w1a4 = np.zeros((C, 512), np.float32)
    p_arr = np.arange(128)
    for c4 in range(4):
        rows = 4 * (p_arr % 16) + c4
        w1a4[:, c4 * 128:(c4 + 1) * 128] = W1a[rows, :].T
    w1cT2 = np.zeros((C, 256), np.float32)
    w1cT2[:, 0:64] = W1c.T
    w1cT2[:, 192:256] = W1c.T
    W2T = W2.T
    w2blk = np.zeros((128, 128), np.float32)
    w2blk[:64, :64] = W2T
    w2blk[64:, 64:] = W2T
    e4 = [np.zeros((128, 512), np.float32) for _ in range(2)]
    for r in range(128):
        r64 = r % 64
        u, j = r64 // 16, r64 % 16
        up, s = u // 2, u % 2
        for c4 in range(4):
            e4[up][r, c4 * 128 + 4 * j + c4 + 64 * s] = 1.0
    self64 = np.zeros((128, C), np.float32)
    for s in range(2):
        self64[np.arange(C) + 64 * s, np.arange(C)] = 1.0
    repl64 = np.zeros((C, 128), np.float32)
    for s in range(2):
        repl64[np.arange(C), np.arange(C) + 64 * s] = 1.0
    gbm = np.ascontiguousarray(
        np.stack([np.asarray(gamma1, np.float32), np.asarray(beta1, np.float32),
                  np.asarray(gamma2, np.float32), np.asarray(beta2, np.float32)],
                 axis=1))
    base = {
        "w1a4": w1a4, "w1cT2": w1cT2,
        "w2blk": w2blk.astype(bf), "w2T32": np.ascontiguousarray(W2T),
        "e4up0": e4[0].astype(bf), "e4up1": e4[1].astype(bf),
        "self64": self64, "repl64": repl64,
        "gb": gbm,
    }
    return [
        dict(base, pts=np.ascontiguousarray(pts[b])) for b in range(n_cores)
    ]


def kernel(points, W1, gamma1, beta1, W2, gamma2, beta2, **run_kwargs):
    nc = _get_nc(B)
    in_maps = make_in_maps(points, W1, gamma1, beta1, W2, gamma2, beta2, B)
    res = run_bass_kernel_spmd(nc, in_maps, core_ids=list(range(B)), **run_kwargs)
    outs = []
    for b in range(B):
        O = np.asarray(res.results[b]["out_t"], np.float32)  # [128, 2048]
        O4 = O.reshape(2, 64, 4, 512)          # [s, c, kp, m]
        outs.append(O4.transpose(2, 0, 3, 1).reshape(N, C))
    kernel.last_results = res
    return np.stack(outs, axis=0).astype(np.float32)
